# revision 62
# baseline (speedup 1.0000x reference)
"""ClsMixAttention Trainium2 Bass kernel.

Strategy: data-parallel over batch across 8 NeuronCores (8 batches/core, no
collectives).  Host-side: tokens permuted to [temporal(128) | cls | spatial(256)]
and x transposed to (C, N) per batch so every matmul streams with the
contraction dim on partitions.  Device-side per batch:
  qT/kT   : (h*d, tok) layout via Wqkv-stationary matmuls  (fp32r)
  v       : (tok, h*d) layout with an interleaved ones-column per head
  S^T     : per head-pair (row-packed K=64 at bases 0/64), keys chunked
            [128,128,128,1]; temporal queries only need key-chunk 0
  P^T     : exp on ScalarE with fused 0.125 scale, written straight to SBUF
  PV      : M=65 matmuls ([v | ones] stationary) -> PV rows 0:64 and the
            softmax denominator in row 64 of the same psum tile
  norm    : denominators PE-broadcast via an E-matrix matmul, DVE
            reciprocal_approx_fast, multiply fused with the evacuation
  proj    : transposed output (c_out, tok); bias folded into the evac as a
            per-partition tensor_scalar_add

Key throughput decisions (vs the naive pipeline):
  - PSUM is split into three independent single-bank slot pools (S^T
    stream x3, qkv/v/bc stream x2, PV-accum + proj x3 = 8 banks) so the
    exp-paced S^T rotation never gates the independent qkv/proj matmuls.
  - The tail key/token (385 = 3*128 + 1) S^T is merged across the head
    pair into one 128-contraction matmul via a block-diagonal [128, 33]
    stationary (head B routed through partition 32 for alignment).
  - Stages are software-pipelined per batch: stretch(b) = attention(b)
    interleaved with qkv(b+1) and proj(b-1).

Token dim padded to 386 on-chip (fp32r matmuls need an even moving dim).
Host gathers (B,768,385) transposed outputs, untransposes and unpermutes.
"""

import os
import sys
import numpy as np

if "/opt/trn_rl_repo" not in sys.path:
    sys.path.insert(0, "/opt/trn_rl_repo")

# The kernel executes through the axon PJRT backend; a JAX_PLATFORMS=cpu pin
# (used by some harnesses for the reference) would hide the NeuronCores.
if "jax" not in sys.modules and "axon" not in os.environ.get("JAX_PLATFORMS", "axon"):
    os.environ.pop("JAX_PLATFORMS", None)

B, N, C = 64, 385, 768
H, D = 12, 64
NT = 128          # temporal tokens (t_h*t_w*(1+online_size))
P = 128
KC = 6            # C / 128 contraction chunks
SCALE = 0.125     # D ** -0.5
NCORES = 8
BPC = B // NCORES
NW = 386          # padded token width (fp32r needs even moving dim)
KCH = [(0, 128), (128, 256), (256, 384), (384, 385)]   # key/token chunks

# token permutation: [temporal (orig 1..128), cls (orig 0), spatial (129..384)]
PERM = np.concatenate([np.arange(1, 1 + NT), [0], np.arange(1 + NT, N)])
INV_PERM = np.argsort(PERM)

_CACHE = {}
VARIANT = "v1"
BUFS_ST, BUFS_QV, BUFS_PV = 3, 2, 3
QK_EVAC_ACT = True
RB_BUFS = 2
DN_ACT = False
PT_MEMSET = True
V_EVAC_ACT = True
PROJ_EVAC_ACT = True
MP_ORDER0 = [0, 1, 2, 3, 4, 5]
OUTST_BUFS = 4
NPOOL_BUFS = 2
EXP_HIPRI = False
NORM_HIPRI = False


def build_nc(bpc=BPC):
    import concourse.bacc as bacc
    import concourse.mybir as mybir
    import concourse.tile as tile

    dt = mybir.dt
    f32 = dt.float32
    R = dt.float32r
    AF = mybir.ActivationFunctionType

    nc = bacc.Bacc("TRN2", target_bir_lowering=False, debug=False)
    xt_d = nc.dram_tensor("xt", [bpc, C, N], f32, kind="ExternalInput")
    wqkv_d = nc.dram_tensor("wqkv", [C, 3 * C], f32, kind="ExternalInput")
    wproj_d = nc.dram_tensor("wproj", [C, C], f32, kind="ExternalInput")
    bproj_d = nc.dram_tensor("bproj", [1, C], f32, kind="ExternalInput")
    out_d = nc.dram_tensor("out", [bpc, C, N], f32, kind="ExternalOutput")

    with tile.TileContext(nc) as tc:
        with (
            tc.tile_pool(name="wpool", bufs=1) as wp,
            tc.tile_pool(name="per_b", bufs=1) as bp,
            tc.tile_pool(name="norm", bufs=NPOOL_BUFS) as npool,
            tc.tile_pool(name="outst", bufs=OUTST_BUFS) as outp,
            tc.tile_pool(name="ps2", bufs=BUFS_ST, space="PSUM") as ps2,
            tc.tile_pool(name="ps3", bufs=BUFS_QV, space="PSUM") as ps3,
            tc.tile_pool(name="ps1", bufs=BUFS_PV, space="PSUM") as ps1,
        ):
            # -------------- persistent double buffers --------------
            xt_sb = [bp.tile([P, KC * NW], R, tag=f"xt{i}", name=f"xt_sb{i}") for i in range(2)]
            qk_sb = [bp.tile([P, 12 * NW], R, tag=f"qk{i}", name=f"qk_sb{i}") for i in range(2)]
            v_sb = [bp.tile([P, 4 * 780], R, tag=f"v{i}", name=f"v_sb{i}") for i in range(2)]
            # pts slots: kc0 holds 2 heads x 386 queries; kc1-3 hold 2 x 258
            # (cs queries only) -- 4 independent slots, no within-pair reuse
            PTB = [0, 772, 772 + 516, 772 + 2 * 516]
            PTW = [NW, 258, 258, 258]
            pt_sb = [bp.tile([P, 772 + 3 * 516], R, tag=f"pt{i}", name=f"pt_sb{i}") for i in range(2)]
            aot_sb = [bp.tile([P, KC * NW], R, tag=f"aot{i}", name=f"aot_sb{i}") for i in range(2)]
            dn_sb = [bp.tile([33, NW], R, tag=f"dn{i}", name=f"dn_sb{i}") for i in range(2)]
            kt3_sb = [bp.tile([P, 6 * 33], R, tag=f"kt3{i}", name=f"kt3_sb{i}") for i in range(2)]
            # memset order matters: the first qkv matmul waits on the xt pad
            # memset, so it must not queue behind the wide pt/v clears
            for t in xt_sb:
                # pad column (token 385) stays zero forever
                nc.vector.memset(
                    t[:, :].rearrange("p (k n) -> p k n", n=NW)[:, :, N:NW].bitcast(dt.uint32), 0
                )
            for t in kt3_sb:
                nc.vector.memset(t[:, :].bitcast(dt.uint32), 0)
            for t in dn_sb:
                nc.vector.memset(t[:, :].bitcast(dt.uint32), 0)
            for t in v_sb:
                # ones column after each head's 64 v-columns (denominator trick)
                nc.vector.memset(
                    t[:, :].rearrange("p (c h e) -> p c h e", h=12, e=65)[:, :, :, 64:65].bitcast(dt.uint32),
                    0x3F800000,
                )
            if PT_MEMSET:
                for t in pt_sb:
                    nc.vector.memset(t[:, :].bitcast(dt.uint32), 0)

            # ---------------- weight tiles (loaded in prologue) ----------------
            wqkv_sb = wp.tile([P, KC * 3 * C], R)       # [128, 13824]
            wproj_sb = wp.tile([P, KC * C], R)          # [128, 4608]
            bproj_pc = wp.tile([P, KC], f32)
            # E-matrix for PE denominator broadcast: rows 0/32 select head A/B
            e_bc = wp.tile([33, P], R)
            nc.vector.memset(e_bc[:, :].bitcast(dt.uint32), 0)
            nc.vector.memset(e_bc[0:1, 0:64].bitcast(dt.uint32), 0x3F800000)
            nc.vector.memset(e_bc[32:33, 64:128].bitcast(dt.uint32), 0x3F800000)

            def load_xt(b):
                xb = xt_sb[b % 2]
                xv = xt_d[b].rearrange("(k p) n -> k p n", p=P)
                for kc in range(KC):
                    nc.sync.dma_start(
                        out=xb[:, kc * NW : kc * NW + N], in_=xv[kc].bitcast(R)
                    )

            def qk_half(b, mc):
                """q/k projection column block mc (0..11): out qk_sb block mc."""
                xb = xt_sb[b % 2]
                qk = qk_sb[b % 2]
                ps = ps3.tile([P, 512], f32, tag="qv", name="ps_qk")
                for kc in range(KC):
                    nc.tensor.matmul(
                        ps[:, 0:NW],
                        wqkv_sb[:, kc * 2304 + mc * P : kc * 2304 + (mc + 1) * P],
                        xb[:, kc * NW : (kc + 1) * NW],
                        start=(kc == 0),
                        stop=(kc == KC - 1),
                    )
                if QK_EVAC_ACT:
                    nc.scalar.activation(qk[:, mc * NW : (mc + 1) * NW], ps[:, 0:NW], AF.Copy, scale=1.0)
                else:
                    nc.vector.tensor_copy(qk[:, mc * NW : (mc + 1) * NW], ps[:, 0:NW])
                if mc >= 6:
                    # pack the tail key (token 384) of both heads into the
                    # block-diagonal [128, 33] stationary for the merged S kc3
                    kt3 = kt3_sb[b % 2]
                    col = (mc - 6) * 33
                    nc.vector.tensor_copy(kt3[0:64, col : col + 1], ps[0:64, 384:385])
                    nc.vector.tensor_copy(kt3[64:128, col + 32 : col + 33], ps[64:128, 384:385])

            def qk_set(b, mp):
                qk_half(b, 2 * mp)
                qk_half(b, 2 * mp + 1)

            def v_half(b, tci, nh):
                """v token-chunk tci, head-half nh: out v_sb rows [t0:t1]."""
                xb = xt_sb[b % 2]
                vb = v_sb[b % 2]
                t0, t1 = KCH[tci]
                tw = t1 - t0
                ps = ps3.tile([P, 512], f32, tag="qv", name="ps_v")
                for kc in range(KC):
                    nc.tensor.matmul(
                        ps[0:tw, 0:384],
                        xb[:, kc * NW + t0 : kc * NW + t1],
                        wqkv_sb[:, kc * 2304 + 1536 + nh * 384 : kc * 2304 + 1536 + (nh + 1) * 384],
                        start=(kc == 0),
                        stop=(kc == KC - 1),
                    )
                if V_EVAC_ACT:
                    nc.scalar.activation(
                        vb[0:tw, tci * 780 + nh * 390 : tci * 780 + (nh + 1) * 390]
                        .rearrange("p (h e) -> p h e", e=65)[:, :, 0:64],
                        ps[0:tw, 0:384].rearrange("p (h e) -> p h e", e=64),
                        AF.Copy,
                        scale=1.0,
                    )
                else:
                    nc.vector.tensor_copy(
                        vb[0:tw, tci * 780 + nh * 390 : tci * 780 + (nh + 1) * 390]
                        .rearrange("p (h e) -> p h e", e=65)[:, :, 0:64],
                        ps[0:tw, 0:384].rearrange("p (h e) -> p h e", e=64),
                    )
                if tci == 3:
                    # duplicate the tail token's v for odd heads onto
                    # partition 32 (merged-kc3 PV reads head B there)
                    for j in (1, 3, 5):
                        h = 6 * nh + j
                        nc.vector.tensor_copy(
                            vb[32:33, tci * 780 + h * 65 : tci * 780 + h * 65 + 64],
                            ps[0:1, j * 64 : (j + 1) * 64],
                        )

            def v_set(b, tci):
                v_half(b, tci, 0)
                v_half(b, tci, 1)

            def s_chunks(b, p, kcis):
                """S^T + exp for head pair p, key chunks `kcis` -> pts slots."""
                qk = qk_sb[b % 2]
                pts = pt_sb[p % 2]
                qoff = p * NW
                koff = (6 + p) * NW
                for kci in kcis:
                    k0, k1 = KCH[kci]
                    kw = k1 - k0
                    base, w = PTB[kci], PTW[kci]
                    # query col range: kc0 serves all queries, others cs only
                    q0, qn = (0, NW) if kci == 0 else (128, 258)
                    if kci == 3:
                        # merged tail-key chunk: block-diagonal [128, 33]
                        # stationary computes both heads in one matmul
                        # (head A -> out row 0, head B -> out row 32)
                        st = ps2.tile([P, 512], f32, tag="st", name="ps_st")
                        nc.tensor.matmul(
                            st[0:33, 0:qn],
                            kt3_sb[b % 2][:, p * 33 : (p + 1) * 33],
                            qk[:, qoff + q0 : qoff + q0 + qn],
                            start=True,
                            stop=True,
                        )
                        ctx = tc.high_priority() if EXP_HIPRI else None
                        if ctx:
                            with ctx:
                                nc.scalar.activation(
                                    pts[0:33, base : base + qn], st[0:33, 0:qn], AF.Exp, scale=SCALE
                                )
                        else:
                            nc.scalar.activation(
                                pts[0:33, base : base + qn], st[0:33, 0:qn], AF.Exp, scale=SCALE
                            )
                        continue
                    for hh in range(2):
                        st = ps2.tile([P, 512], f32, tag="st", name="ps_st")
                        nc.tensor.matmul(
                            st[0:kw, 0:qn],
                            qk[hh * 64 : (hh + 1) * 64, koff + k0 : koff + k1],
                            qk[hh * 64 : (hh + 1) * 64, qoff + q0 : qoff + q0 + qn],
                            start=True,
                            stop=True,
                        )
                        if EXP_HIPRI:
                            with tc.high_priority():
                                nc.scalar.activation(
                                    pts[0:kw, base + hh * w : base + hh * w + qn],
                                    st[0:kw, 0:qn], AF.Exp, scale=SCALE,
                                )
                        else:
                            nc.scalar.activation(
                                pts[0:kw, base + hh * w : base + hh * w + qn],
                                st[0:kw, 0:qn], AF.Exp, scale=SCALE,
                            )

            def s_group(b, p):
                s_chunks(b, p, range(4))

            def pv_group(b, p):
                """PV accumulation for head pair p -> pva/pvb psum tiles."""
                vb = v_sb[b % 2]
                pts = pt_sb[p % 2]
                pva = ps1.tile([P, 512], f32, tag="u", name="ps_pva")
                pvb = ps1.tile([P, 512], f32, tag="u", name="ps_pvb")
                pvs = (pva, pvb)
                for kci, (k0, k1) in enumerate(KCH):
                    kw = k1 - k0
                    base, w = PTB[kci], PTW[kci]
                    q0, qn = (0, NW) if kci == 0 else (128, 258)
                    for hh in range(2):
                        # lhsT = [v_h | ones] (65 cols) -> rows 0:64 PV, row 64 denom
                        pp = hh * 32 if kci == 3 else 0
                        rhs = (
                            pts[pp : pp + 1, base : base + qn]
                            if kci == 3
                            else pts[0:kw, base + hh * w : base + hh * w + qn]
                        )
                        nc.tensor.matmul(
                            pvs[hh][0:65, q0 : q0 + qn],
                            vb[pp : pp + kw, kci * 780 + (2 * p + hh) * 65 : kci * 780 + (2 * p + hh) * 65 + 65],
                            rhs,
                            start=(kci == 0),
                            stop=(kci == 3),
                        )
                return pva, pvb

            def norm_group(b, p, pva, pvb):
                if NORM_HIPRI:
                    with tc.high_priority():
                        return norm_group_inner(b, p, pva, pvb)
                return norm_group_inner(b, p, pva, pvb)

            def norm_group_inner(b, p, pva, pvb):
                """normalize: PE-broadcast denoms via E-matrix, reciprocal, multiply."""
                aot = aot_sb[b % 2]
                dn2 = dn_sb[p % 2]
                nc.vector.tensor_copy(dn2[0:1, :], pva[64:65, 0:NW])
                if DN_ACT:
                    nc.scalar.activation(dn2[32:33, :], pvb[64:65, 0:NW], AF.Copy, scale=1.0)
                else:
                    nc.vector.tensor_copy(dn2[32:33, :], pvb[64:65, 0:NW])
                bc = ps3.tile([P, 512], f32, tag="qv", name="ps_bc")
                nc.tensor.matmul(bc[:, 0:NW], e_bc[:, :], dn2[:, :], start=True, stop=True)
                rb = npool.tile([P, NW], f32, tag="rb", name="rb", bufs=RB_BUFS)
                nc.vector.reciprocal_approx_fast(rb[:, :], bc[:, 0:NW])
                nc.vector.tensor_mul(
                    aot[0:64, p * NW : (p + 1) * NW], pva[0:64, 0:NW], rb[0:64, :]
                )
                nc.vector.tensor_mul(
                    aot[64:128, p * NW : (p + 1) * NW], pvb[0:64, 0:NW], rb[64:128, :]
                )

            def proj_mc(b, mc):
                """projection output chunk mc (128 c_out rows)."""
                aot = aot_sb[b % 2]
                ps = ps1.tile([P, 512], f32, tag="u", name="ps_u")
                for kc in range(KC):
                    nc.tensor.matmul(
                        ps[:, 0:NW],
                        wproj_sb[:, kc * C + mc * P : kc * C + (mc + 1) * P],
                        aot[:, kc * NW : (kc + 1) * NW],
                        start=(kc == 0),
                        stop=(kc == KC - 1),
                    )
                ot = outp.tile([P, N], f32, tag="ot", name="ot")
                if PROJ_EVAC_ACT:
                    nc.scalar.activation(ot[:, :], ps[:, 0:N], AF.Identity, bias=bproj_pc[:, mc : mc + 1], scale=1.0)
                else:
                    nc.vector.tensor_scalar_add(ot[:, :], ps[:, 0:N], bproj_pc[:, mc : mc + 1])
                nc.sync.dma_start(
                    out=out_d[b].rearrange("(k p) n -> k p n", p=P)[mc],
                    in_=ot[:, :],
                )

            # ---------------- prologue: interleaved weight + x DMA ----------------
            wv = wqkv_d[:].rearrange("(k p) c -> k p c", p=P)
            xv0 = xt_d[0].rearrange("(k p) n -> k p n", p=P)
            # pair xt(0) chunks with the q-quarter weight chunks so the first
            # qk_set can start as soon as chunk 0 of each lands
            for kc in range(KC):
                nc.sync.dma_start(
                    out=xt_sb[0][:, kc * NW : kc * NW + N], in_=xv0[kc].bitcast(R)
                )
                nc.sync.dma_start(
                    out=wqkv_sb[:, kc * 2304 : kc * 2304 + 576],
                    in_=wv[kc, :, 0:576].bitcast(R),
                )
            for q4 in range(1, 4):
                for kc in range(KC):
                    nc.sync.dma_start(
                        out=wqkv_sb[:, kc * 2304 + q4 * 576 : kc * 2304 + (q4 + 1) * 576],
                        in_=wv[kc, :, q4 * 576 : (q4 + 1) * 576].bitcast(R),
                    )
            wpv = wproj_d[:].rearrange("(k p) c -> k p c", p=P)
            for kc in range(KC):
                nc.sync.dma_start(
                    out=wproj_sb[:, kc * C : (kc + 1) * C], in_=wpv[kc].bitcast(R)
                )
            # bias in partition-major layout: value (mc*128+p) at [p, mc]
            nc.sync.dma_start(
                out=bproj_pc[:, :],
                in_=bproj_d[0].rearrange("(k p) -> p k", p=P),
            )
            load_xt(1)
            # qkv(0) sequential (DMA-paced pipeline head)
            for mp in MP_ORDER0:
                qk_set(0, mp)
            for tci in range(4):
                v_set(0, tci)

            # ---------------- software-pipelined stretches ----------------
            # stretch b: attention(b) + qkv(b+1) + proj(b-1)
            variant = VARIANT

            def stretch(b):
                has_next = b + 1 < bpc
                if b + 2 < bpc:
                    load_xt(b + 2)
                if variant == "v1":
                    s_group(b, 0)
                    if has_next:
                        qk_set(b + 1, 0)
                    for p in range(5):
                        s_group(b, p + 1)
                        pva, pvb = pv_group(b, p)
                        norm_group(b, p, pva, pvb)
                        if has_next:
                            qk_set(b + 1, p + 1)
                        if b > 0:
                            proj_mc(b - 1, p)
                    pva, pvb = pv_group(b, 5)
                    norm_group(b, 5, pva, pvb)
                    if has_next:
                        for tci in range(4):
                            v_set(b + 1, tci)
                    if b > 0:
                        proj_mc(b - 1, 5)
                elif variant == "v2":
                    # split S around PV; fillers between S chunk pairs
                    s_group(b, 0)
                    if has_next:
                        qk_set(b + 1, 0)
                    for p in range(6):
                        if p + 1 < 6:
                            s_chunks(b, p + 1, [0, 1])
                        pva, pvb = pv_group(b, p)
                        if p + 1 < 6:
                            s_chunks(b, p + 1, [2, 3])
                        norm_group(b, p, pva, pvb)
                        if has_next:
                            if p < 5:
                                qk_set(b + 1, p + 1)
                            else:
                                for tci in range(4):
                                    v_set(b + 1, tci)
                        if b > 0:
                            proj_mc(b - 1, p)
                elif variant == "v3":
                    # v2 but fillers interleaved at half-set granularity
                    s_group(b, 0)
                    if has_next:
                        qk_half(b + 1, 0)
                        qk_half(b + 1, 1)
                    for p in range(6):
                        if p + 1 < 6:
                            s_chunks(b, p + 1, [0, 1])
                        pva, pvb = pv_group(b, p)
                        if has_next:
                            if p < 5:
                                qk_half(b + 1, 2 * p + 2)
                        if p + 1 < 6:
                            s_chunks(b, p + 1, [2, 3])
                        norm_group(b, p, pva, pvb)
                        if has_next:
                            if p < 5:
                                qk_half(b + 1, 2 * p + 3)
                            else:
                                for tci in range(4):
                                    v_set(b + 1, tci)
                        if b > 0:
                            proj_mc(b - 1, p)
                else:
                    raise ValueError(variant)

            for b in range(bpc):
                stretch(b)
            for mc in range(KC):
                proj_mc(bpc - 1, mc)

    nc.compile()
    return nc


def _prep_inputs(x, Wqkv, Wproj, bproj):
    x = np.asarray(x, dtype=np.float32)
    xt = np.ascontiguousarray(x[:, PERM, :].transpose(0, 2, 1))   # (B, C, N)
    wqkv = np.ascontiguousarray(np.asarray(Wqkv, dtype=np.float32))
    wproj = np.ascontiguousarray(np.asarray(Wproj, dtype=np.float32))
    bp = np.ascontiguousarray(np.asarray(bproj, dtype=np.float32).reshape(1, C))
    in_maps = []
    for i in range(NCORES):
        in_maps.append({
            "xt": np.ascontiguousarray(xt[i * BPC : (i + 1) * BPC]),
            "wqkv": wqkv,
            "wproj": wproj,
            "bproj": bp,
        })
    return in_maps


def _postprocess(results):
    outs = [results[i]["out"] for i in range(NCORES)]          # (BPC, C, N) each
    out_t = np.concatenate(outs, axis=0)                       # (B, C, N)
    out = out_t.transpose(0, 2, 1)[:, INV_PERM, :]             # (B, N, C)
    return np.ascontiguousarray(out)


def run(inputs, trace=False):
    from concourse.bass_utils import run_bass_kernel_spmd

    if "nc" not in _CACHE:
        _CACHE["nc"] = build_nc(BPC)
    nc = _CACHE["nc"]
    in_maps = _prep_inputs(inputs["x"], inputs["Wqkv"], inputs["Wproj"], inputs["bproj"])
    res = run_bass_kernel_spmd(nc, in_maps, list(range(NCORES)), trace=trace)
    return _postprocess(res.results), res


def kernel(x, Wqkv, Wproj, bproj, t_h=8, t_w=8, s_h=16, s_w=16, online_size=1, num_heads=12, **_):
    assert int(t_h) * int(t_w) * (1 + int(online_size)) == NT
    assert int(s_h) * int(s_w) == N - 1 - NT
    assert int(num_heads) == H
    out, _res = run({"x": x, "Wqkv": Wqkv, "Wproj": Wproj, "bproj": bproj})
    return out
